# revision 63
# baseline (speedup 1.0000x reference)
"""Fused sparse-attention kernel for Trainium2 — 8-core SPMD, data-parallel over batch.

Reference computation (per call, two calls: (V, r_i) and (T, r_t)):
    q      = x @ Wq.T + bq                      # [b,256,768]
    k      = r @ Wk.T + bk                      # [b,8,256,768]
    v      = r @ Wv.T + bv
    S      = (q @ k.T) / sqrt(768)              # [b,8,256,256]
    P      = softmax(S, -1)
    out    = mean_k( pool16(P @ v) )            # pool16: avg over groups of 16 q rows

Algebraic restructuring used here:
  1. softmax is shift-invariant => the bk bias term (constant along s) drops.
     S = q' @ r.T with q' = x @ Wqk + bqk,  Wqk = Wq.T @ Wk * scale (host-precomputed).
     This removes the [b,8,256,768]x[768,768] k-projection entirely.
  2. pool16 is linear => apply it to P before the value matmul:
     pool16(P @ v) = (pool16 P) @ v.  16x fewer FLOPs in the AV matmul.
  3. Wv/bv projection commutes with the k-mean and the pool:
     out = ( mean_k (pool16 P) @ r ) @ Wv.T + bv.  One value projection per batch
     instead of per (batch, k); bv survives unscaled because pooled probs rows sum to 1.
  4. pooling + softmax normalization + k-mean are folded into one tiny matmul:
     Ppooled.T[s, p] = sum_q E[q, s] * M[q, p] / (16*8*rowsum[q]), with
     E = exp(S) and M the 0/1 pool-bin scatter matrix (host constant).

Total ~76 GFLOP instead of ~435 GFLOP.

Per-core dataflow (fp32 accumulation everywhere):
  - x, r loaded with SWDGE cast f32->bf16.
  - r.T tiles are produced with PE transposes (bf16) and evicted to SBUF as
    fp8e4 (cast in the DVE copy). q'.T is evicted from PSUM as fp8e4 with a
    x8 scale folded into Wqk host-side (keeps q' out of the fp8 subnormal
    range); the exp activation descales by 1/8.
  - scores run as fp8 DoubleRow matmuls (2 contraction k-tiles per
    instruction, 2x PE throughput). The AV matmul and q-projection stay bf16:
    fp8 there pushes the end-to-end error over the 2e-2 gate (measured
    4.3e-2 in a numpy bit-model); scores-only fp8 lands at ~1.1e-2.
  - exp on ScalarE reads scores straight from PSUM and emits the softmax
    denominator via accum_out in the same pass.

Scheduling notes (all measured on HW, each worth 5-20us):
  - Engine queues are in-order, so a whole prologue emitted before a kloop
    stalls the PE queue on its internal DVE/Act round-trips; prologues are
    split into pieces drained one per k-iteration instead.
  - PSUM pools are per-stage: one shared rotating pool chains a kloop's
    transposes behind the NEXT prologue's buffer recycling (22us head
    stall when the next x tile is still in flight).
  - GPSIMD cannot read PSUM, and costs ~900ns fixed per instruction, so
    all PSUM evictions stay on DVE/Act; small SBUF ops are split between
    DVE (recip) and Act (w-scale, ppT, half the U eviction) to balance.
  - r transposes run 2 candidates ahead of the scores; the first two bc's
    r loads are issued at k-pair granularity because the ramp is paced by
    DMA arrival.
"""

import numpy as np
import ml_dtypes

B, K, S, SQ, D = 32, 8, 256, 256, 768
NCORES = 8
BL = B // NCORES          # batches per core
P16 = 16                  # pooled length
NBC = 2 * BL              # (call, batch) units per core
DC = D // 128             # 6 chunks of the feature dim
KH = 4                    # candidates per load/transpose half-group
QS = 8.0                  # q' prescale: keeps fp8 q' out of subnormals
WQS = 32.0                # Wqk prescale: Wqk entries (~1e-3) need x32 for fp8
BF16 = ml_dtypes.bfloat16
F8 = ml_dtypes.float8_e4m3fn

_cache = {}


def _build_program():
    import concourse.bass as bass
    import concourse.bacc as bacc
    import concourse.tile as tile
    import concourse.mybir as mybir

    f32 = mybir.dt.float32
    bf16 = mybir.dt.bfloat16
    f8 = mybir.dt.float8e4
    ts = bass.ts
    AF = mybir.ActivationFunctionType
    DR = mybir.MatmulPerfMode.DoubleRow

    nc = bacc.Bacc("TRN2", target_bir_lowering=False, debug=False)

    xv = nc.dram_tensor("xv", [BL, SQ, D], f32, kind="ExternalInput").ap()
    xt = nc.dram_tensor("xt", [BL, SQ, D], f32, kind="ExternalInput").ap()
    rv = nc.dram_tensor("rv", [BL, K, S, D], f32, kind="ExternalInput").ap()
    rt = nc.dram_tensor("rt", [BL, K, S, D], f32, kind="ExternalInput").ap()
    wqk = nc.dram_tensor("wqk", [D, D], bf16, kind="ExternalInput").ap()
    wvt = nc.dram_tensor("wvt", [D, D], bf16, kind="ExternalInput").ap()
    bqk = nc.dram_tensor("bqk", [128, DC], f32, kind="ExternalInput").ap()
    bvc = nc.dram_tensor("bvc", [128, DC], f32, kind="ExternalInput").ap()
    msk = nc.dram_tensor("msk", [SQ, P16], bf16, kind="ExternalInput").ap()
    idn = nc.dram_tensor("idn", [128, 128], bf16, kind="ExternalInput").ap()
    outT = nc.dram_tensor("outT", [D, NBC * P16], f32, kind="ExternalOutput").ap()

    with tile.TileContext(nc) as tc:
        with (
            tc.tile_pool(name="const", bufs=1) as const,
            tc.tile_pool(name="persist", bufs=1) as persist,
            tc.tile_pool(name="bcpool", bufs=2) as bcp,
            tc.tile_pool(name="xpool", bufs=3) as xpool,
            tc.tile_pool(name="rpool", bufs=4) as rpool,
            tc.tile_pool(name="rtpool", bufs=4) as rtpool,
            tc.tile_pool(name="pair", bufs=6) as pair,
            # PSUM budget: 8 banks of 2KB. scores 2 + U-accum 2 + kloop
            # transposes 2 + prologue 1 + pooled-probs 1. Separate pools per
            # pipeline stage: a shared rotating pool chains the kloop's
            # transposes behind the next prologue's PSUM recycling (measured
            # as a 22us head stall and ~0.7us/bc steady-state gaps).
            tc.tile_pool(name="ps_scores", bufs=2, space="PSUM") as ps_sc,
            tc.tile_pool(name="ps_u", bufs=1, space="PSUM") as ps_up,
            tc.tile_pool(name="ps_pro", bufs=1, space="PSUM") as ps_pro,
            tc.tile_pool(name="ps_tr", bufs=2, space="PSUM") as ps_tr,
            tc.tile_pool(name="ps_pp", bufs=1, space="PSUM") as ps_pp,
        ):
            # ---- constants on the critical path (idn -> first transposes,
            # wqk/bqk -> first q-projection, msk -> first kloop) ----
            idn_sb = const.tile([128, 128], bf16)
            nc.sync.dma_start(idn_sb[:], idn[:])
            bqk_sb = const.tile([128, DC], f32)
            nc.sync.dma_start(bqk_sb[:], bqk[:])
            msk_sb = const.tile([128, 2, P16], bf16)
            nc.sync.dma_start(msk_sb[:], msk.rearrange("(t p) m -> p t m", p=128))
            # wqk (1.2MB) is issued by the schedule after x0 so the first x
            # transposes start as early as possible
            wqk_sb = const.tile([128, DC, D], bf16)

            def issue_wqk():
                nc.sync.dma_start(wqk_sb[:], wqk.rearrange("(c p) d -> p c d", p=128))
            # epilogue-only constants; their DMAs are issued after the first
            # r loads so they don't steal HBM bandwidth from the head
            wvt_sb = const.tile([128, DC, D], bf16)
            bvc_sb = const.tile([128, DC], f32)

            def issue_epilogue_consts():
                nc.sync.dma_start(wvt_sb[:], wvt.rearrange("(c p) d -> p c d", p=128))
                nc.sync.dma_start(bvc_sb[:], bvc[:])

            # transposed, Wv-unprojected pooled outputs for every (call, batch)
            uT_all = persist.tile([128, DC, NBC, P16], bf16)
            U_all = persist.tile([48, NBC, D], bf16)

            def dram_xr(bc):
                call, b = bc // BL, bc % BL
                return (xv, xt)[call][b], (rv, rt)[call][b]

            x_tiles, r_tiles, rT_tiles, qT_tiles = {}, {}, {}, {}

            def issue_x_load(bc):
                x_dram, _ = dram_xr(bc)
                x_sb = xpool.tile([128, 2, D], bf16, tag="xsb")
                x_re = x_dram.rearrange("(t p) d -> p t d", p=128)
                # per-half loads: the first transposes only need t=0
                for t in range(2):
                    nc.gpsimd.dma_start(out=x_sb[:, t], in_=x_re[:, t])
                x_tiles[bc] = x_sb

            def issue_r_loads(bc, hs=(0, 1), pairs=False):
                _, r_dram = dram_xr(bc)
                if bc not in r_tiles:
                    r_tiles[bc], rT_tiles[bc] = {}, {}
                for h in hs:
                    r_re = r_dram[h * KH : (h + 1) * KH].rearrange(
                        "k (t p) d -> p t k d", p=128
                    )
                    r_sb = rpool.tile([128, 2, KH, D], bf16, tag="rsb")
                    rT_sb = rtpool.tile([128, KH, DC, S], f8, tag="rtsb")
                    if pairs:
                        # k-pair granularity for the ramp: the first kloops
                        # are paced by r arrival, so smaller chunks unblock
                        # the transposes sooner
                        for i in range(KH // 2):
                            for t in range(2):
                                nc.gpsimd.dma_start(
                                    out=r_sb[:, t, 2 * i : 2 * i + 2],
                                    in_=r_re[:, t, 2 * i : 2 * i + 2],
                                )
                    else:
                        for t in range(2):
                            nc.gpsimd.dma_start(out=r_sb[:, t], in_=r_re[:, t])
                    r_tiles[bc][h] = r_sb
                    rT_tiles[bc][h] = rT_sb

            def prologue_pieces(bc):
                """x.T transposes + q'.T projection as a list of thunks.

                The PE queue is in-order, so emitting a whole prologue at
                once parks ~48 PE instructions (with internal DVE/Act
                round-trip stalls) in front of the next kloop's work.
                Instead the kloop drains one piece per k-iteration, so the
                engine round-trips hide behind kloop compute.
                """
                x_sb = x_tiles.pop(bc)
                xT_sb = bcp.tile([128, DC, SQ], bf16, tag="xT")
                # q-projection stays bf16: fp8 here lands the end-to-end
                # error at 1.9e-2, too close to the 2e-2 gate
                qT_sb = bcp.tile([128, DC, SQ], f8, tag="qT")
                qT_tiles[bc] = qT_sb

                def xpiece(t):
                    pst = ps_pro.tile([128, DC, 128], bf16, tag="pro")
                    for c in range(DC):
                        nc.tensor.transpose(
                            pst[:, c], x_sb[:, t, ts(c, 128)], idn_sb[:]
                        )
                    nc.vector.tensor_copy(xT_sb[:, :, ts(t, 128)], pst[:])

                def qpiece(co):
                    psq = ps_pro.tile([128, SQ], f32, tag="pro")
                    for ci in range(DC):
                        nc.tensor.matmul(
                            psq[:],
                            lhsT=wqk_sb[:, ci, ts(co, 128)],
                            rhs=xT_sb[:, ci, :],
                            start=(ci == 0),
                            stop=(ci == DC - 1),
                        )
                    nc.scalar.activation(
                        qT_sb[:, co, :], psq[:], AF.Identity,
                        bias=bqk_sb[:, co : co + 1], scale=1.0,
                    )

                return [lambda t=t: xpiece(t) for t in range(2)] + [
                    lambda co=co: qpiece(co) for co in range(DC)
                ]

            def transpose_unit(bc, k, t):
                # r.T (one s-half) for candidate k on the PE (no xbar: its
                # transfer time blocks all other DMA traffic)
                r_sb = r_tiles[bc][k // KH]
                rT_sb = rT_tiles[bc][k // KH]
                kl = k % KH
                psr = ps_tr.tile([128, DC, 128], bf16, tag="tr")
                for c in range(DC):
                    nc.tensor.transpose(
                        psr[:, c], r_sb[:, t, kl, ts(c, 128)], idn_sb[:]
                    )
                nc.vector.tensor_copy(rT_sb[:, kl, :, ts(t, 128)], psr[:])

            # U accumulator, double-buffered by bc parity at partition
            # offsets 0/32 within one 2-bank tile (tile_position requires
            # 32-aligned output partition bases; extra partitions are free)
            psu2 = ps_up.tile([48, 2, 512], f32)
            # pooled-probs PSUM, rotated manually by k parity (a 2-buf pool
            # slot would still be bank-granular)
            psp2 = ps_pp.tile([128, 2, 2, P16], f32)

            def kloop(bc, pro):
                qT_sb = qT_tiles.pop(bc)
                uoff = 32 * (bc % 2)
                psu = psu2[uoff : uoff + 16]
                for kt in range(4):
                    transpose_unit(bc, kt // 2, kt % 2)
                for k in range(K):
                    if k + 2 < K:
                        transpose_unit(bc, k + 2, 0)
                        transpose_unit(bc, k + 2, 1)
                    if pro:
                        pro.pop(0)()
                    kl = k % KH
                    r_sb = r_tiles[bc][k // KH]
                    rT_sb = rT_tiles[bc][k // KH]
                    pss = ps_sc.tile([128, 2, S], f32)
                    for qc in range(2):
                        for cp in range(DC // 2):
                            nc.tensor.matmul(
                                pss[:, qc],
                                lhsT=qT_sb[:, 2 * cp : 2 * cp + 2, ts(qc, 128)],
                                rhs=rT_sb[:, kl, 2 * cp : 2 * cp + 2, :],
                                start=(cp == 0),
                                stop=(cp == DC // 2 - 1),
                                perf_mode=DR,
                            )
                    E_sb = pair.tile([128, 2, S], bf16)
                    rs_sb = pair.tile([128, 2], f32)
                    ri_sb = pair.tile([128, 2], f32)
                    w_sb = pair.tile([128, 2, P16], bf16)
                    for qc in range(2):
                        nc.scalar.activation(
                            E_sb[:, qc], pss[:, qc], AF.Exp,
                            accum_out=rs_sb[:, qc : qc + 1],
                            scale=1.0 / QS,
                        )
                        nc.vector.reciprocal(
                            ri_sb[:, qc : qc + 1], rs_sb[:, qc : qc + 1]
                        )
                    # w = msk * (1/rowsum) on Act (scale takes a per-partition
                    # AP), keeping DVE free for the big eviction casts
                    for qc in range(2):
                        nc.scalar.activation(
                            w_sb[:, qc], msk_sb[:, qc], AF.Identity,
                            scale=ri_sb[:, qc : qc + 1],
                        )
                    # pooled probs (transposed): Pp.T[s,p] = sum_q E[q,s] w[q,p]
                    psp = psp2[:, k % 2]
                    for sc in range(2):
                        for qc in range(2):
                            nc.tensor.matmul(
                                psp[:, sc],
                                lhsT=E_sb[:, qc, ts(sc, 128)],
                                rhs=w_sb[:, qc],
                                start=(qc == 0),
                                stop=(qc == 1),
                            )
                    ppT_sb = pair.tile([128, 2, P16], bf16)
                    for sc in range(2):
                        nc.scalar.activation(ppT_sb[:, sc], psp[:, sc], AF.Identity)
                    # U += Pp @ r   (accumulate over k in PSUM)
                    for sc in range(2):
                        st = k == 0 and sc == 0
                        sp = k == K - 1 and sc == 1
                        nc.tensor.matmul(
                            psu[:, 0, :],
                            lhsT=ppT_sb[:, sc],
                            rhs=r_sb[:, sc, kl, 0:512],
                            start=st, stop=sp, skip_group_check=True,
                        )
                        nc.tensor.matmul(
                            psu[:, 1, 0:256],
                            lhsT=ppT_sb[:, sc],
                            rhs=r_sb[:, sc, kl, 512:768],
                            start=st, stop=sp, skip_group_check=True,
                        )
                while pro:
                    pro.pop(0)()
                del r_tiles[bc], rT_tiles[bc]
                # ---- evict U, then build its transpose right away so the
                # epilogue only has the final projection left ----
                ua = U_all[uoff : uoff + 16]
                nc.vector.tensor_copy(ua[:, bc, 0:512], psu[:, 0, :])
                nc.scalar.activation(
                    ua[:, bc, 512:768], psu[:, 1, 0:256], AF.Identity
                )
                for c in range(DC):
                    pst2 = ps_pro.tile([128, P16], bf16, tag="pro")
                    nc.tensor.transpose(
                        pst2[:], ua[:, bc, ts(c, 128)],
                        idn_sb[uoff : uoff + 16, uoff : uoff + 16],
                    )
                    nc.vector.tensor_copy(uT_all[:, c, bc, :], pst2[:])

            # software-pipelined schedule: r loads run two bc ahead; the
            # next bc's prologue pieces drain inside the current kloop.
            issue_x_load(0)
            issue_wqk()
            issue_r_loads(0, pairs=True)
            issue_x_load(1)
            # dependency-free transposes of the identity fill the DMA-bound
            # head and ramp the PE p-state before real work lands (~40
            # instructions, consumed by nothing; they only need idn)
            for _ in range(40):
                warm = ps_tr.tile([128, DC, 128], bf16, tag="tr")
                nc.tensor.transpose(warm[:, 0], idn_sb[:], idn_sb[:])
            for p in prologue_pieces(0):
                p()
            issue_r_loads(1, pairs=True)
            issue_epilogue_consts()
            for bc in range(NBC):
                if bc + 2 < NBC:
                    issue_x_load(bc + 2)
                pro = prologue_pieces(bc + 1) if bc + 1 < NBC else []
                if bc + 2 < NBC:
                    issue_r_loads(bc + 2)
                kloop(bc, pro)

            # ---- final: out.T = Wv @ U.T + bv, all (call,batch) columns at once ----
            fT_sb = persist.tile([128, DC, NBC * P16], f32)
            for co in range(DC):
                psf = ps_tr.tile([128, NBC * P16], f32, tag="tr")
                for ci in range(DC):
                    nc.tensor.matmul(
                        psf[:],
                        lhsT=wvt_sb[:, ci, ts(co, 128)],
                        rhs=uT_all[:, ci],
                        start=(ci == 0),
                        stop=(ci == DC - 1),
                    )
                nc.vector.tensor_scalar_add(fT_sb[:, co], psf[:], bvc_sb[:, co : co + 1])
            nc.sync.dma_start(
                out=outT.rearrange("(c p) n -> p c n", p=128), in_=fT_sb[:]
            )

    nc.compile()
    return nc


def _host_weights(Wq, bq, Wk, Wv, bv):
    scale = QS / np.sqrt(np.float32(D))
    Wqk = (Wq.astype(np.float32).T @ Wk.astype(np.float32)) * scale
    bqk = (bq.astype(np.float32) @ Wk.astype(np.float32)) * scale
    mask = np.zeros((SQ, P16), np.float32)
    mask[np.arange(SQ), np.arange(SQ) // P16] = 1.0 / (P16 * K)
    return {
        "wqk": Wqk.astype(BF16),
        "wvt": np.ascontiguousarray(Wv.astype(np.float32).T).astype(BF16),
        "bqk": np.ascontiguousarray(bqk.reshape(DC, 128).T),
        "bvc": np.ascontiguousarray(bv.astype(np.float32).reshape(DC, 128).T),
        "msk": mask.astype(BF16),
        "idn": np.eye(128, dtype=BF16),
    }


def make_in_maps(V, T, r_i, r_t, Wq, bq, Wk, bk, Wv, bv):
    w = _host_weights(Wq, bq, Wk, Wv, bv)
    in_maps = []
    for c in range(NCORES):
        sl = slice(c * BL, (c + 1) * BL)
        m = dict(w)
        m["xv"] = np.ascontiguousarray(V[sl], dtype=np.float32)
        m["xt"] = np.ascontiguousarray(T[sl], dtype=np.float32)
        m["rv"] = np.ascontiguousarray(r_i[sl], dtype=np.float32)
        m["rt"] = np.ascontiguousarray(r_t[sl], dtype=np.float32)
        in_maps.append(m)
    return in_maps


def assemble(outTs):
    """outTs: list of per-core outT [D, NBC*P16] f32 -> (T_to_T, V_to_V)."""
    Ts, Vs = [], []
    for a in outTs:
        a = a.reshape(D, 2, BL, P16)
        Vs.append(np.ascontiguousarray(a[:, 0].transpose(1, 2, 0)))
        Ts.append(np.ascontiguousarray(a[:, 1].transpose(1, 2, 0)))
    return (
        np.concatenate(Ts, axis=0).astype(np.float32),
        np.concatenate(Vs, axis=0).astype(np.float32),
    )


def get_program():
    if "nc" not in _cache:
        _cache["nc"] = _build_program()
    return _cache["nc"]


def kernel(V, T, r_i, r_t, Wq, bq, Wk, bk, Wv, bv):
    from concourse import bass_utils

    nc = get_program()
    in_maps = make_in_maps(V, T, r_i, r_t, Wq, bq, Wk, bk, Wv, bv)
    res = bass_utils.run_bass_kernel_spmd(nc, in_maps, core_ids=list(range(NCORES)))
    return assemble([r["outT"] for r in res.results])

